# revision 10
# baseline (speedup 1.0000x reference)
"""DCGRU cell on 8 Trainium2 NeuronCores — fp8 DoubleRow diffusion.

Sharding: data-parallel over batch B=64 -> 8 batches per core; supports and
weights replicated; no collectives.

Key structure (vs the bf16 chained baseline):
  * Host precomputes s0^2, s1^2 (input-only transform), so each gconv's four
    diffusion mats {s0 x, s0^2 x, s1 x, s1^2 x} are independent and share one
    stationary x-tile across 4 consecutive matmuls (measured 1.8x PE rate vs
    bf16 on HW via fp8 DoubleRow: contract 256 per MM at ~257 ns).
  * Supports are scaled by 64 before fp8 quantization (s^2 entries ~6e-4 sit
    in fp8e4 subnormal range); the 1/64 is folded into projection weights.
  * gconv1 x-column layout [hx-block (512, b-major) | inputs-block (16)]:
    a diffusion psum ci-tile [128, n] is exactly 2 batches x 64 hx features,
    already feature-major — its bf16 SBUF evacuation is consumed DIRECTLY as
    the projection moving operand (no DRAM stash round-trip for hx mats).
  * The tiny inputs-part mats (shared by both gconvs — inputs don't change)
    go through a small DRAM stash and ride the m0 projection matmul as 8
    extra contract rows (mv0 extended to 74 rows).
  * Projection/stash path stays bf16; fp8 only enters the diffusion operands.
  * xg2 (gconv2's graph-major r*hx) is built by DMA-engine transposes + Pool
    fp8 casts — the PE does only matmuls.
"""
import sys

import ml_dtypes
import numpy as np

sys.path.insert(0, "/opt/trn_rl_repo")

from concourse import bacc, mybir, tile  # noqa: E402
from concourse.bass_utils import run_bass_kernel_spmd  # noqa: E402

B = 64
N = 4096
U = 64
IN_DIM = 2
F = U + IN_DIM          # 66, feature order [hx, inputs]
FE = F + 4 * IN_DIM     # 74: mv0 rows = x0 feature-major + 4 in-part mats
NCORES = 8
BL = B // NCORES        # 8
HB = U * BL             # 512 hx-part width
IB = IN_DIM * BL        # 16 inputs-part width
FB = F * BL             # 528
NCH = 8                 # n-chunks
CHW = N // NCH          # 512
JT2 = 16                # j-pair tiles (256 contract each)
SSCALE = 64.0           # support fp8 pre-scale (folded into proj weights)

F32 = mybir.dt.float32
BF16 = mybir.dt.bfloat16
FP8 = mybir.dt.float8e4
DR = mybir.MatmulPerfMode.DoubleRow
SIGMOID = mybir.ActivationFunctionType.Sigmoid
TANH = mybir.ActivationFunctionType.Tanh

CTS1 = [(0, 128), (128, 128), (256, 128), (384, 128), (512, 16)]
CTS2 = [(0, 128), (128, 128), (256, 128), (384, 128)]

_BUILD_CACHE = {}
_NAME_N = [0]


def _nm(base):
    _NAME_N[0] += 1
    return f"{base}_{_NAME_N[0]}"


def _emit_gconv_chunk(nc, pools, g, nch, xg, sT_d, stash_i):
    """Diffusion mats for one n-chunk: 4 DR matmuls per (ci, jt2) sharing the
    stationary xg slice; psum accumulated over jt2, evacuated to bf16 SBUF
    tiles (returned for direct use as projection moving operands); the
    inputs-part (g=0 ci 4) goes to the small DRAM stash instead."""
    pst, pmm, pstage, pstin = (
        pools["pst"], pools["pmm"], pools["pstage"], pools["pstin"],
    )
    n0 = nch * CHW
    cts = CTS1 if g == 0 else CTS2
    sts = []
    for jt2 in range(JT2):
        row = []
        for m in range(4):
            st = pst.tile([128, 2, CHW], FP8, tag="st", name=_nm("st"))
            nc.sync.dma_start(
                st[:],
                sT_d[m, jt2 * 256:(jt2 + 1) * 256, n0:n0 + CHW].rearrange(
                    "(t p) n -> p t n", p=128
                ),
            )
            row.append(st)
        sts.append(row)
    stgs = []
    for ci, (c0, cw) in enumerate(cts):
        pss = [pmm.tile([128, CHW], F32, tag="mm", name=_nm("ps"))
               for _ in range(4)]
        for jt2 in range(JT2):
            for m in range(4):
                nc.tensor.matmul(
                    pss[m][0:cw, :],
                    xg[:, jt2, :, c0:c0 + cw],
                    sts[jt2][m][:],
                    start=(jt2 == 0),
                    stop=(jt2 == JT2 - 1),
                    perf_mode=DR,
                )
        row = []
        for m in range(4):
            if ci < 4:
                pair = []
                for hh in range(2):
                    stg = pstage.tile([U, CHW], BF16, tag="stage",
                                      name=_nm("stg"))
                    if (m + hh) % 2 == 0:
                        nc.vector.tensor_copy(
                            stg[:, :], pss[m][hh * U:(hh + 1) * U, :])
                    else:
                        nc.scalar.copy(
                            stg[:, :], pss[m][hh * U:(hh + 1) * U, :])
                    pair.append(stg)
                row.append(pair)
            else:
                sti = pstin.tile([16, CHW], BF16, tag="stin", name=_nm("sti"))
                if m % 2 == 0:
                    nc.vector.tensor_copy(sti[:, :], pss[m][0:16, :])
                else:
                    nc.scalar.copy(sti[:, :], pss[m][0:16, :])
                nc.scalar.dma_start(stash_i[nch][:, m, :], sti[:, :])
        stgs.append(row)
    return stgs


def _emit_proj_chunk(nc, pools, g, nch, w, bias, xt0c_d, xt02_l, stgs,
                     stash_i, u_l, hxT_d, out_d, xg2):
    """Projection + activation (+ gating for g=1) for one n-chunk. The hx-mat
    moving operands are the in-SBUF evacuation tiles; inputs-part mats ride
    mv0 rows 66:74 through the m0 matmul."""
    pmov, pproj, psig, ps2, pgate, pxs = (
        pools["pmov"], pools["pproj"], pools["psig"], pools["ps2"],
        pools["pgate"], pools["pxs"],
    )
    O = 128 if g == 0 else 64
    n0 = nch * CHW
    for b in range(BL):
        mv0 = pmov.tile([FE, CHW], BF16, tag="mov0", bufs=4, name=_nm("mv0"))
        if g == 0:
            nc.scalar.dma_start(mv0[0:F, :],
                                xt0c_d[b * F:(b + 1) * F, n0:n0 + CHW])
        else:
            nc.scalar.dma_start(mv0[0:F, :],
                                xt02_l[nch][b * F:(b + 1) * F, :])
        nc.scalar.dma_start(
            mv0[F:FE, :],
            stash_i[nch][b * IN_DIM:(b + 1) * IN_DIM, :, :].rearrange(
                "f m n -> (f m) n"),
        )
        pp = pproj.tile([128, CHW], F32, tag="pp", name=_nm("pp"))
        nc.tensor.matmul(pp[0:O, :], w[:, 0, :], mv0[:], start=True,
                         stop=False)
        for m in range(1, 5):
            nc.tensor.matmul(
                pp[0:O, :], w[0:U, m, :],
                stgs[b // 2][m - 1][b % 2][:, :],
                start=False, stop=(m == 4),
            )
        if g == 0:
            sig = psig.tile([128, CHW], BF16, tag="sig", name=_nm("sig"))
            nc.scalar.activation(sig[:], pp[:], SIGMOID, bias=bias[:])
            s2 = ps2.tile([F, CHW], BF16, tag="s2", name=_nm("s2"))
            # rows 0:64 = r * hx, rows 64:66 = inputs (for gconv2 projection)
            nc.vector.tensor_mul(s2[0:U, :], sig[0:U, :], mv0[0:U, :])
            nc.vector.tensor_copy(s2[U:F, :], mv0[U:F, :])
            nc.scalar.dma_start(xt02_l[nch][b * F:(b + 1) * F, :], s2[:])
            nc.scalar.dma_start(u_l[nch][b, :, :], sig[U:128, :])
            for blk in range(4):
                xs = pxs.tile([128, U], BF16, tag="xs", name=_nm("xs"))
                nc.scalar.dma_start_transpose(
                    xs[:], s2[0:U, blk * 128:(blk + 1) * 128]
                )
                nb = nch * 4 + blk
                nc.gpsimd.tensor_copy(
                    xg2[:, nb // 2, nb % 2, b * U:(b + 1) * U], xs[:]
                )
        else:
            ct = pgate.tile([U, CHW], F32, tag="ct", name=_nm("ct"))
            nc.scalar.activation(ct[:], pp[0:U, :], TANH, bias=bias[:])
            ut = pgate.tile([U, CHW], BF16, tag="ut", name=_nm("ut"))
            nc.sync.dma_start(ut[:], u_l[nch][b, :, :])
            hxt = pgate.tile([U, CHW], F32, tag="hxt", name=_nm("hxt"))
            nc.sync.dma_start(hxt[:], hxT_d[b, :, n0:n0 + CHW])
            t1 = pgate.tile([U, CHW], F32, tag="t1", name=_nm("t1"))
            nc.vector.tensor_sub(t1[:], hxt[:], ct[:])
            nc.vector.tensor_mul(t1[:], ut[:], t1[:])
            nc.vector.tensor_add(t1[:], t1[:], ct[:])
            nc.scalar.dma_start(out_d[b, :, n0:n0 + CHW], t1[:])


def _build(reps=1):
    if reps in _BUILD_CACHE:
        return _BUILD_CACHE[reps]
    nc = bacc.Bacc("TRN2", target_bir_lowering=False, debug=False)

    sT_d = nc.dram_tensor("sT", [4, N, N], FP8, kind="ExternalInput").ap()
    xg0_d = nc.dram_tensor("xg0", [N, FB], FP8, kind="ExternalInput").ap()
    xt0c_d = nc.dram_tensor("xt0c", [FB, N], BF16, kind="ExternalInput").ap()
    hxT_d = nc.dram_tensor("hxT", [BL, U, N], F32, kind="ExternalInput").ap()
    w1_d = nc.dram_tensor("w1", [FE, 5, 2 * U], BF16,
                          kind="ExternalInput").ap()
    b1_d = nc.dram_tensor("b1", [2 * U, 1], F32, kind="ExternalInput").ap()
    w2_d = nc.dram_tensor("w2", [FE, 5, U], BF16, kind="ExternalInput").ap()
    b2_d = nc.dram_tensor("b2", [U, 1], F32, kind="ExternalInput").ap()
    out_d = nc.dram_tensor("outT", [BL, U, N], F32, kind="ExternalOutput").ap()

    with tile.TileContext(nc) as tc:
        with (
            tc.tile_pool(name="dram", bufs=1, space="DRAM") as dram,
            tc.tile_pool(name="pw", bufs=1) as pw,
            tc.tile_pool(name="pxg", bufs=1) as pxg,
            tc.tile_pool(name="pst", bufs=88) as pst,
            tc.tile_pool(name="pstage", bufs=40) as pstage,
            tc.tile_pool(name="pstin", bufs=4) as pstin,
            tc.tile_pool(name="pmov", bufs=3) as pmov,
            tc.tile_pool(name="ps2", bufs=2) as ps2,
            tc.tile_pool(name="psig", bufs=2) as psig,
            tc.tile_pool(name="pgate", bufs=2) as pgate,
            tc.tile_pool(name="pxs", bufs=4) as pxs,
            tc.tile_pool(name="pmm", bufs=6, space="PSUM") as pmm,
            tc.tile_pool(name="pproj", bufs=2, space="PSUM") as pproj,
        ):
            pools = dict(
                pst=pst, pstage=pstage, pstin=pstin, pmov=pmov, ps2=ps2,
                psig=psig, pgate=pgate, pmm=pmm, pproj=pproj, pxs=pxs,
            )
            stash_i = [dram.tile([IB, 4, CHW], BF16, name=_nm("sti"))
                       for _ in range(NCH)]
            xt02_l = [dram.tile([FB, CHW], BF16, name=_nm("xt02"))
                      for _ in range(NCH)]
            u_l = [dram.tile([BL, U, CHW], BF16, name=_nm("ud"))
                   for _ in range(NCH)]

            w1 = pw.tile([FE, 5, 2 * U], BF16, tag="w1")
            nc.sync.dma_start(w1[:], w1_d)
            w2 = pw.tile([FE, 5, U], BF16, tag="w2")
            nc.sync.dma_start(w2[:], w2_d)
            b1 = pw.tile([2 * U, 1], F32, tag="b1")
            nc.sync.dma_start(b1[:], b1_d)
            b2 = pw.tile([U, 1], F32, tag="b2")
            nc.sync.dma_start(b2[:], b2_d)

            for _rep in range(reps):
                xg0 = pxg.tile([128, JT2, 2, FB], FP8, tag="xg0",
                               name=_nm("xg0"))
                nc.sync.dma_start(
                    xg0[:], xg0_d.rearrange("(j t p) c -> p j t c", p=128, t=2)
                )
                xg2 = pxg.tile([128, JT2, 2, HB], FP8, tag="xg2",
                               name=_nm("xg2"))

                for nch in range(NCH):
                    stgs = _emit_gconv_chunk(nc, pools, 0, nch, xg0, sT_d,
                                             stash_i)
                    _emit_proj_chunk(nc, pools, 0, nch, w1, b1, xt0c_d,
                                     xt02_l, stgs, stash_i, u_l, hxT_d,
                                     out_d, xg2)
                for nch in range(NCH):
                    stgs = _emit_gconv_chunk(nc, pools, 1, nch, xg2, sT_d,
                                             stash_i)
                    _emit_proj_chunk(nc, pools, 1, nch, w2, b2, xt0c_d,
                                     xt02_l, stgs, stash_i, u_l, hxT_d,
                                     out_d, None)

    nc.compile()
    _BUILD_CACHE[reps] = nc
    return nc


def _host_prep(inputs, hx, supports, ru_weights, ru_biases, gconv_weights,
               gconv_biases):
    """Per-core input maps. gconv1 x column layout: [hx-block 512 | in-block
    16] (b-major inside each block); proj/feature-major layout keeps the
    [hx(64), in(2)] interleave per batch; mv0 weight slot carries the
    inputs-part mat weights as 8 extra rows."""
    s = np.asarray(supports, dtype=np.float32)
    s2_0 = s[0] @ s[0]
    s2_1 = s[1] @ s[1]
    sT4 = np.stack([s[0].T, s2_0.T, s[1].T, s2_1.T]) * SSCALE
    sT4 = np.ascontiguousarray(sT4).astype(ml_dtypes.float8_e4m3)

    hx3 = np.asarray(hx, np.float32).reshape(B, N, U)
    in3 = np.asarray(inputs, np.float32).reshape(B, N, IN_DIM)
    x = np.concatenate([hx3, in3], axis=2)  # [B, N, F] order [hx, in]

    # weight rows (f_orig, m), f_orig order [in, hx] -> permute to [hx, in];
    # Chebyshev fold (x2 = 2*y2 - x0, x4 = 2*y4 - x0) + support 1/SSCALE.
    # Packed layout [FE, 5, o]: slot 0 rows 0:66 = W0' (applied to x0
    # feature-major), rows 66:74 = in-part weights of m=1..4 (m-major);
    # slots 1..4 rows 0:64 = hx-part weights.
    def prep_w(wt, o):
        wr = np.asarray(wt, np.float32).reshape(F, 5, o)
        wr = np.concatenate([wr[IN_DIM:], wr[:IN_DIM]], axis=0).copy()
        wr[:, 0] = wr[:, 0] - wr[:, 2] - wr[:, 4]
        wr[:, 2] = 2.0 * wr[:, 2]
        wr[:, 4] = 2.0 * wr[:, 4]
        wr[:, 1:] /= SSCALE
        wp = np.zeros((FE, 5, o), np.float32)
        wp[0:F, 0] = wr[:, 0]
        for m in range(1, 5):
            for fi in range(IN_DIM):
                wp[F + fi * 4 + (m - 1), 0] = wr[U + fi, m]
            wp[0:U, m] = wr[0:U, m]
        return np.ascontiguousarray(wp).astype(ml_dtypes.bfloat16)

    w1 = prep_w(ru_weights, 2 * U)
    w2 = prep_w(gconv_weights, U)
    b1 = np.ascontiguousarray(
        np.asarray(ru_biases, np.float32).reshape(2 * U, 1))
    b2 = np.ascontiguousarray(
        np.asarray(gconv_biases, np.float32).reshape(U, 1))

    in_maps = []
    for c in range(NCORES):
        xb = x[c * BL:(c + 1) * BL]  # [BL, N, F]
        # spmm graph-major: [hx-block (b-major, 512) | in-block (16)]
        xh = xb[:, :, :U].transpose(1, 0, 2).reshape(N, HB)
        xi = xb[:, :, U:].transpose(1, 0, 2).reshape(N, IB)
        xg0 = np.ascontiguousarray(
            np.concatenate([xh, xi], axis=1)).astype(ml_dtypes.float8_e4m3)
        # proj feature-major: rows b*66+f, f order [hx, in]
        xt0c = np.ascontiguousarray(
            xb.transpose(0, 2, 1).reshape(FB, N)).astype(ml_dtypes.bfloat16)
        hxT = np.ascontiguousarray(xb[:, :, :U].transpose(0, 2, 1))
        in_maps.append({
            "sT": sT4, "xg0": xg0, "xt0c": xt0c, "hxT": hxT,
            "w1": w1, "b1": b1, "w2": w2, "b2": b2,
        })
    return in_maps


def kernel(inputs, hx, supports, ru_weights, ru_biases, gconv_weights,
           gconv_biases):
    nc = _build()
    in_maps = _host_prep(
        inputs, hx, supports, ru_weights, ru_biases, gconv_weights,
        gconv_biases
    )
    res = run_bass_kernel_spmd(nc, in_maps, list(range(NCORES))).results
    outs = []
    for c in range(NCORES):
        outT = res[c]["outT"]  # [BL, U, N]
        outs.append(outT.transpose(0, 2, 1).reshape(BL, N * U))
    return np.concatenate(outs, axis=0).astype(np.float32)


# revision 11
# speedup vs baseline: 1.0560x; 1.0560x over previous
"""DCGRU cell on 8 Trainium2 NeuronCores — fp8 DoubleRow diffusion.

Sharding: data-parallel over batch B=64 -> 8 batches per core; supports and
weights replicated; no collectives.

Key structure (vs the bf16 chained baseline):
  * Host precomputes s0^2, s1^2 (input-only transform), so each gconv's four
    diffusion mats {s0 x, s0^2 x, s1 x, s1^2 x} are independent and share one
    stationary x-tile across 4 consecutive matmuls (measured 1.8x PE rate vs
    bf16 on HW via fp8 DoubleRow: contract 256 per MM at ~257 ns).
  * Supports are scaled by 64 before fp8 quantization (s^2 entries ~6e-4 sit
    in fp8e4 subnormal range); the 1/64 is folded into projection weights.
  * gconv1 x-column layout [hx-block (512, b-major) | inputs-block (16)]:
    a diffusion psum ci-tile [128, n] is exactly 2 batches x 64 hx features,
    already feature-major — its bf16 SBUF evacuation is consumed DIRECTLY as
    the projection moving operand (no DRAM stash round-trip for hx mats).
  * The tiny inputs-part mats (shared by both gconvs — inputs don't change)
    go through a small DRAM stash and ride the m0 projection matmul as 8
    extra contract rows (mv0 extended to 74 rows).
  * Projection/stash path stays bf16; fp8 only enters the diffusion operands.
  * xg2 (gconv2's graph-major r*hx) is built by DMA-engine transposes + Pool
    fp8 casts — the PE does only matmuls.
"""
import sys

import ml_dtypes
import numpy as np

sys.path.insert(0, "/opt/trn_rl_repo")

from concourse import bacc, mybir, tile  # noqa: E402
from concourse.bass_utils import run_bass_kernel_spmd  # noqa: E402

B = 64
N = 4096
U = 64
IN_DIM = 2
F = U + IN_DIM          # 66, feature order [hx, inputs]
FE = F + 4 * IN_DIM     # 74: mv0 rows = x0 feature-major + 4 in-part mats
NCORES = 8
BL = B // NCORES        # 8
HB = U * BL             # 512 hx-part width
IB = IN_DIM * BL        # 16 inputs-part width
FB = F * BL             # 528
NCH = 8                 # n-chunks
CHW = N // NCH          # 512
JT2 = 16                # j-pair tiles (256 contract each)
SSCALE = 64.0           # support fp8 pre-scale (folded into proj weights)

F32 = mybir.dt.float32
BF16 = mybir.dt.bfloat16
FP8 = mybir.dt.float8e4
DR = mybir.MatmulPerfMode.DoubleRow
SIGMOID = mybir.ActivationFunctionType.Sigmoid
TANH = mybir.ActivationFunctionType.Tanh

CTS1 = [(0, 128), (128, 128), (256, 128), (384, 128), (512, 16)]
CTS2 = [(0, 128), (128, 128), (256, 128), (384, 128)]

_BUILD_CACHE = {}
_NAME_N = [0]


def _nm(base):
    _NAME_N[0] += 1
    return f"{base}_{_NAME_N[0]}"


def _emit_gconv_chunk(nc, pools, g, nch, xg, sT_d, stash_i):
    """Diffusion mats for one n-chunk: 4 DR matmuls per (ci, jt2) sharing the
    stationary xg slice; psum accumulated over jt2, evacuated to bf16 SBUF
    tiles (returned for direct use as projection moving operands); the
    inputs-part (g=0 ci 4) goes to the small DRAM stash instead."""
    pst, pmm, pstage, pstin = (
        pools["pst"], pools["pmm"], pools["pstage"], pools["pstin"],
    )
    n0 = nch * CHW
    cts = CTS1 if g == 0 else CTS2
    sts = []
    for jt2 in range(JT2):
        row = []
        for m in range(4):
            st = pst.tile([128, 2, CHW], FP8, tag="st", name=_nm("st"))
            nc.sync.dma_start(
                st[:],
                sT_d[m, jt2 * 256:(jt2 + 1) * 256, n0:n0 + CHW].rearrange(
                    "(t p) n -> p t n", p=128
                ),
            )
            row.append(st)
        sts.append(row)
    stgs = []
    for ci, (c0, cw) in enumerate(cts):
        pss = [pmm.tile([128, CHW], F32, tag="mm", name=_nm("ps"))
               for _ in range(4)]
        for jt2 in range(JT2):
            for m in range(4):
                nc.tensor.matmul(
                    pss[m][0:cw, :],
                    xg[:, jt2, :, c0:c0 + cw],
                    sts[jt2][m][:],
                    start=(jt2 == 0),
                    stop=(jt2 == JT2 - 1),
                    perf_mode=DR,
                )
        row = []
        for m in range(4):
            if ci < 4:
                pair = []
                for hh in range(2):
                    stg = pstage.tile([U, CHW], BF16, tag="stage",
                                      name=_nm("stg"))
                    if (m + hh) % 2 == 0:
                        nc.vector.tensor_copy(
                            stg[:, :], pss[m][hh * U:(hh + 1) * U, :])
                    else:
                        nc.scalar.copy(
                            stg[:, :], pss[m][hh * U:(hh + 1) * U, :])
                    pair.append(stg)
                row.append(pair)
            else:
                sti = pstin.tile([16, CHW], BF16, tag="stin", name=_nm("sti"))
                if m % 2 == 0:
                    nc.vector.tensor_copy(sti[:, :], pss[m][0:16, :])
                else:
                    nc.scalar.copy(sti[:, :], pss[m][0:16, :])
                nc.scalar.dma_start(stash_i[nch][:, m, :], sti[:, :])
        stgs.append(row)
    return stgs


def _emit_proj_chunk(nc, pools, g, nch, w, bias, xt0c_d, xt02_l, stgs,
                     stash_i, u_l, hxT_d, out_d, xg2):
    """Projection + activation (+ gating for g=1) for one n-chunk. The hx-mat
    moving operands are the in-SBUF evacuation tiles; inputs-part mats ride
    mv0 rows 66:74 through the m0 matmul."""
    pmov, pproj, psig, ps2, pgate, pxs = (
        pools["pmov"], pools["pproj"], pools["psig"], pools["ps2"],
        pools["pgate"], pools["pxs"],
    )
    O = 128 if g == 0 else 64
    n0 = nch * CHW
    for b in range(BL):
        mv0 = pmov.tile([FE, CHW], BF16, tag="mov0", bufs=4, name=_nm("mv0"))
        if g == 0:
            nc.scalar.dma_start(mv0[0:F, :],
                                xt0c_d[b * F:(b + 1) * F, n0:n0 + CHW])
        else:
            nc.scalar.dma_start(mv0[0:F, :],
                                xt02_l[nch][b * F:(b + 1) * F, :])
        nc.scalar.dma_start(
            mv0[F:FE, :],
            stash_i[nch][b * IN_DIM:(b + 1) * IN_DIM, :, :].rearrange(
                "f m n -> (f m) n"),
        )
        pp = pproj.tile([128, CHW], F32, tag="pp", name=_nm("pp"))
        nc.tensor.matmul(pp[0:O, :], w[:, 0, :], mv0[:], start=True,
                         stop=False)
        for m in range(1, 5):
            nc.tensor.matmul(
                pp[0:O, :], w[0:U, m, :],
                stgs[b // 2][m - 1][b % 2][:, :],
                start=False, stop=(m == 4),
            )
        if g == 0:
            sig = psig.tile([128, CHW], BF16, tag="sig", name=_nm("sig"))
            nc.scalar.activation(sig[:], pp[:], SIGMOID, bias=bias[:])
            s2 = ps2.tile([F, CHW], BF16, tag="s2", name=_nm("s2"))
            # rows 0:64 = r * hx, rows 64:66 = inputs (for gconv2 projection)
            nc.vector.tensor_mul(s2[0:U, :], sig[0:U, :], mv0[0:U, :])
            nc.vector.tensor_copy(s2[U:F, :], mv0[U:F, :])
            nc.scalar.dma_start(xt02_l[nch][b * F:(b + 1) * F, :], s2[:])
            nc.scalar.dma_start(u_l[nch][b, :, :], sig[U:128, :])
            for blk in range(4):
                xs = pxs.tile([128, U], BF16, tag="xs", name=_nm("xs"))
                nc.scalar.dma_start_transpose(
                    xs[:], s2[0:U, blk * 128:(blk + 1) * 128]
                )
                nb = nch * 4 + blk
                nc.gpsimd.tensor_copy(
                    xg2[:, nb // 2, nb % 2, b * U:(b + 1) * U], xs[:]
                )
        else:
            ct = pgate.tile([U, CHW], F32, tag="ct", name=_nm("ct"))
            nc.scalar.activation(ct[:], pp[0:U, :], TANH, bias=bias[:])
            ut = pgate.tile([U, CHW], BF16, tag="ut", name=_nm("ut"))
            nc.scalar.dma_start(ut[:], u_l[nch][b, :, :])
            hxt = pgate.tile([U, CHW], F32, tag="hxt", name=_nm("hxt"))
            nc.scalar.dma_start(hxt[:], hxT_d[b, :, n0:n0 + CHW])
            t1 = pgate.tile([U, CHW], F32, tag="t1", name=_nm("t1"))
            nc.vector.tensor_sub(t1[:], hxt[:], ct[:])
            nc.vector.tensor_mul(t1[:], ut[:], t1[:])
            nc.vector.tensor_add(t1[:], t1[:], ct[:])
            nc.scalar.dma_start(out_d[b, :, n0:n0 + CHW], t1[:])


def _build(reps=1):
    if reps in _BUILD_CACHE:
        return _BUILD_CACHE[reps]
    nc = bacc.Bacc("TRN2", target_bir_lowering=False, debug=False)

    sT_d = nc.dram_tensor("sT", [4, N, N], FP8, kind="ExternalInput").ap()
    xg0_d = nc.dram_tensor("xg0", [N, FB], FP8, kind="ExternalInput").ap()
    xt0c_d = nc.dram_tensor("xt0c", [FB, N], BF16, kind="ExternalInput").ap()
    hxT_d = nc.dram_tensor("hxT", [BL, U, N], F32, kind="ExternalInput").ap()
    w1_d = nc.dram_tensor("w1", [FE, 5, 2 * U], BF16,
                          kind="ExternalInput").ap()
    b1_d = nc.dram_tensor("b1", [2 * U, 1], F32, kind="ExternalInput").ap()
    w2_d = nc.dram_tensor("w2", [FE, 5, U], BF16, kind="ExternalInput").ap()
    b2_d = nc.dram_tensor("b2", [U, 1], F32, kind="ExternalInput").ap()
    out_d = nc.dram_tensor("outT", [BL, U, N], F32, kind="ExternalOutput").ap()

    with tile.TileContext(nc) as tc:
        with (
            tc.tile_pool(name="dram", bufs=1, space="DRAM") as dram,
            tc.tile_pool(name="pw", bufs=1) as pw,
            tc.tile_pool(name="pxg", bufs=1) as pxg,
            tc.tile_pool(name="pst", bufs=88) as pst,
            tc.tile_pool(name="pstage", bufs=48) as pstage,
            tc.tile_pool(name="pstin", bufs=4) as pstin,
            tc.tile_pool(name="pmov", bufs=3) as pmov,
            tc.tile_pool(name="ps2", bufs=2) as ps2,
            tc.tile_pool(name="psig", bufs=2) as psig,
            tc.tile_pool(name="pgate", bufs=2) as pgate,
            tc.tile_pool(name="pxs", bufs=4) as pxs,
            tc.tile_pool(name="pmm", bufs=6, space="PSUM") as pmm,
            tc.tile_pool(name="pproj", bufs=2, space="PSUM") as pproj,
        ):
            pools = dict(
                pst=pst, pstage=pstage, pstin=pstin, pmov=pmov, ps2=ps2,
                psig=psig, pgate=pgate, pmm=pmm, pproj=pproj, pxs=pxs,
            )
            stash_i = [dram.tile([IB, 4, CHW], BF16, name=_nm("sti"))
                       for _ in range(NCH)]
            xt02_l = [dram.tile([FB, CHW], BF16, name=_nm("xt02"))
                      for _ in range(NCH)]
            u_l = [dram.tile([BL, U, CHW], BF16, name=_nm("ud"))
                   for _ in range(NCH)]

            w1 = pw.tile([FE, 5, 2 * U], BF16, tag="w1")
            nc.sync.dma_start(w1[:], w1_d)
            w2 = pw.tile([FE, 5, U], BF16, tag="w2")
            nc.sync.dma_start(w2[:], w2_d)
            b1 = pw.tile([2 * U, 1], F32, tag="b1")
            nc.sync.dma_start(b1[:], b1_d)
            b2 = pw.tile([U, 1], F32, tag="b2")
            nc.sync.dma_start(b2[:], b2_d)

            for _rep in range(reps):
                xg0 = pxg.tile([128, JT2, 2, FB], FP8, tag="xg0",
                               name=_nm("xg0"))
                nc.sync.dma_start(
                    xg0[:], xg0_d.rearrange("(j t p) c -> p j t c", p=128, t=2)
                )
                xg2 = pxg.tile([128, JT2, 2, HB], FP8, tag="xg2",
                               name=_nm("xg2"))

                for nch in range(NCH):
                    stgs = _emit_gconv_chunk(nc, pools, 0, nch, xg0, sT_d,
                                             stash_i)
                    _emit_proj_chunk(nc, pools, 0, nch, w1, b1, xt0c_d,
                                     xt02_l, stgs, stash_i, u_l, hxT_d,
                                     out_d, xg2)
                for nch in range(NCH):
                    stgs = _emit_gconv_chunk(nc, pools, 1, nch, xg2, sT_d,
                                             stash_i)
                    _emit_proj_chunk(nc, pools, 1, nch, w2, b2, xt0c_d,
                                     xt02_l, stgs, stash_i, u_l, hxT_d,
                                     out_d, None)

    nc.compile()
    _BUILD_CACHE[reps] = nc
    return nc


def _host_prep(inputs, hx, supports, ru_weights, ru_biases, gconv_weights,
               gconv_biases):
    """Per-core input maps. gconv1 x column layout: [hx-block 512 | in-block
    16] (b-major inside each block); proj/feature-major layout keeps the
    [hx(64), in(2)] interleave per batch; mv0 weight slot carries the
    inputs-part mat weights as 8 extra rows."""
    s = np.asarray(supports, dtype=np.float32)
    s2_0 = s[0] @ s[0]
    s2_1 = s[1] @ s[1]
    sT4 = np.stack([s[0].T, s2_0.T, s[1].T, s2_1.T]) * SSCALE
    sT4 = np.ascontiguousarray(sT4).astype(ml_dtypes.float8_e4m3)

    hx3 = np.asarray(hx, np.float32).reshape(B, N, U)
    in3 = np.asarray(inputs, np.float32).reshape(B, N, IN_DIM)
    x = np.concatenate([hx3, in3], axis=2)  # [B, N, F] order [hx, in]

    # weight rows (f_orig, m), f_orig order [in, hx] -> permute to [hx, in];
    # Chebyshev fold (x2 = 2*y2 - x0, x4 = 2*y4 - x0) + support 1/SSCALE.
    # Packed layout [FE, 5, o]: slot 0 rows 0:66 = W0' (applied to x0
    # feature-major), rows 66:74 = in-part weights of m=1..4 (m-major);
    # slots 1..4 rows 0:64 = hx-part weights.
    def prep_w(wt, o):
        wr = np.asarray(wt, np.float32).reshape(F, 5, o)
        wr = np.concatenate([wr[IN_DIM:], wr[:IN_DIM]], axis=0).copy()
        wr[:, 0] = wr[:, 0] - wr[:, 2] - wr[:, 4]
        wr[:, 2] = 2.0 * wr[:, 2]
        wr[:, 4] = 2.0 * wr[:, 4]
        wr[:, 1:] /= SSCALE
        wp = np.zeros((FE, 5, o), np.float32)
        wp[0:F, 0] = wr[:, 0]
        for m in range(1, 5):
            for fi in range(IN_DIM):
                wp[F + fi * 4 + (m - 1), 0] = wr[U + fi, m]
            wp[0:U, m] = wr[0:U, m]
        return np.ascontiguousarray(wp).astype(ml_dtypes.bfloat16)

    w1 = prep_w(ru_weights, 2 * U)
    w2 = prep_w(gconv_weights, U)
    b1 = np.ascontiguousarray(
        np.asarray(ru_biases, np.float32).reshape(2 * U, 1))
    b2 = np.ascontiguousarray(
        np.asarray(gconv_biases, np.float32).reshape(U, 1))

    in_maps = []
    for c in range(NCORES):
        xb = x[c * BL:(c + 1) * BL]  # [BL, N, F]
        # spmm graph-major: [hx-block (b-major, 512) | in-block (16)]
        xh = xb[:, :, :U].transpose(1, 0, 2).reshape(N, HB)
        xi = xb[:, :, U:].transpose(1, 0, 2).reshape(N, IB)
        xg0 = np.ascontiguousarray(
            np.concatenate([xh, xi], axis=1)).astype(ml_dtypes.float8_e4m3)
        # proj feature-major: rows b*66+f, f order [hx, in]
        xt0c = np.ascontiguousarray(
            xb.transpose(0, 2, 1).reshape(FB, N)).astype(ml_dtypes.bfloat16)
        hxT = np.ascontiguousarray(xb[:, :, :U].transpose(0, 2, 1))
        in_maps.append({
            "sT": sT4, "xg0": xg0, "xt0c": xt0c, "hxT": hxT,
            "w1": w1, "b1": b1, "w2": w2, "b2": b2,
        })
    return in_maps


def kernel(inputs, hx, supports, ru_weights, ru_biases, gconv_weights,
           gconv_biases):
    nc = _build()
    in_maps = _host_prep(
        inputs, hx, supports, ru_weights, ru_biases, gconv_weights,
        gconv_biases
    )
    res = run_bass_kernel_spmd(nc, in_maps, list(range(NCORES))).results
    outs = []
    for c in range(NCORES):
        outT = res[c]["outT"]  # [BL, U, N]
        outs.append(outT.transpose(0, 2, 1).reshape(BL, N * U))
    return np.concatenate(outs, axis=0).astype(np.float32)
